# revision 19
# baseline (speedup 1.0000x reference)
"""Trainium2 Bass kernel for nn_BoxMultiHeadedAttention_81312320848177.

Self-contained: kernel(**inputs) takes FULL inputs, shards batch over 8
NeuronCores (2 batches/core), runs a Tile/Bass kernel per core, gathers.

v2 structure (per core, BL=2, N=256, D=1024, H=8, DK=128):
- DMA order: boxes, const blobs (2), Wq, xT transposes, Wk, Wv, Wo.
- Phase A (Ln region): box columns + pairwise ln-deltas (lnd/lnl f32r split).
- no_sync_barrier, then the Sin region: phase B (V33/PU) + 32 (b,gi) geo
  iterations with the QKV projection chunks interleaved (q@t2+, k@t10+, v@t18+).
- Geo per gi: SEL-expansion matmuls (f32r), FRAC range reduction (DVE custom),
  Sin (ACT), packed WgPk/PU matmuls, relu on Pool, transpose, gAT copies
  split DVE/Pool.
- Tail (Exp region): attention per (b,h): scores, exp, geo-mul, denom+attnV
  matmuls, reciprocal-normalize; O-projection in swapped orientation
  (tokens on partitions) so the output needs no transposes; bias via rank-1
  matmul; direct DMA store.
"""
import sys
sys.path.insert(0, '/opt/trn_rl_repo')

import numpy as np
from contextlib import ExitStack

B, N, D, H, DK = 16, 256, 1024, 8, 128
BL = 2                 # batches per core
NCORES = 8
WAVE_LEN = 1000.0
C_ROUND = float(1.5 * 2**23)
TWO_PI = float(2 * np.pi)
INV_SQRT_DK = float(1.0 / np.sqrt(DK))

# const blob column layout (f32 blob: (128, CWF); bf16 blob: (128, CWB))
CF_SEL = 0          # (128, 8, 128) -> cols [0, 1024)
CF_EBC = 1024       # (8, 2, 128) on partitions 0-7 -> cols [1024, 1280)
CF_LAMV = 1280      # (128, 1)
CF_LAM232 = 1281    # (2, 32) partitions 0-1 -> cols [1281, 1313)
CF_SHIFT32 = 1313   # (32, 1) partitions 0-31
CF_BG = 1314        # (1, 8) partition 0
CF_BQ = 1322        # (128, 8)
CF_BK = 1330        # (128, 8)
CWF = 1338

CB_WGPK = 0         # (128, 4, 128) -> cols [0, 512)
CB_BVB = 512        # (128, 1024) -> cols [512, 1536)
CB_ASK = 1536       # (32, 8, 32) partitions 0-31 -> cols [1536, 1792)
CB_BOR = 1792       # (1, 1024) partition 0
CWB = 2816

_BUILD_CACHE = {}


# ------------------------------------------------------------------ host prep

def _lam():
    f = np.arange(8, dtype=np.float64)
    return (100.0 / (2 * np.pi) * WAVE_LEN ** (-f / 8)).astype(np.float32)


def _host_constants(Wg, bg, bf16):
    """Pack all small constants into one f32 blob and one bf16 blob."""
    lam = _lam()
    Wg = np.asarray(Wg, np.float32)

    blocks = [(0, 0), (1, 8), (2, 32), (3, 40)]  # (blk, col0): xs, ys, xc, yc
    WgPk = np.zeros((128, 4, 128), np.float32)
    for blk, col0 in blocks:
        for h in range(H):
            for i16 in range(16):
                m = 16 * h + i16
                for fi in range(8):
                    k = i16 * 8 + fi
                    WgPk[k, blk, m] = Wg[h, col0 + fi]

    A = np.zeros((H, 32, 32), np.float32)
    for h in range(H):
        wsw, wcw = Wg[h, 16:24], Wg[h, 48:56]
        wsh, wch = Wg[h, 24:32], Wg[h, 56:64]
        for fi in range(8):
            A[h, fi, 8 + fi] += wsw[fi]
            A[h, 8 + fi, fi] += -wsw[fi]
            A[h, 8 + fi, 8 + fi] += wcw[fi]
            A[h, fi, fi] += wcw[fi]
            A[h, 16 + fi, 24 + fi] += wsh[fi]
            A[h, 24 + fi, 16 + fi] += -wsh[fi]
            A[h, 24 + fi, 24 + fi] += wch[fi]
            A[h, 16 + fi, 16 + fi] += wch[fi]
    Ask = A.transpose(1, 0, 2)  # (32 k, 8 h, 32 f'): Ask[k, h, f'] = A[h, k, f']

    SEL = np.zeros((128, 8, 128), np.float32)
    for gsub in range(8):
        for ii in range(16):
            for fi in range(8):
                SEL[16 * gsub + ii, gsub, ii * 8 + fi] = 1.0

    EBC = np.zeros((8, 2, 128), np.float32)
    EBC[2, 0, :] = 1.0
    EBC[3, 1, :] = 1.0

    cf = np.zeros((128, CWF), np.float32)
    cf[:, CF_SEL:CF_SEL + 1024] = SEL.reshape(128, 1024)
    cf[:8, CF_EBC:CF_EBC + 256] = EBC.reshape(8, 256)
    cf[:, CF_LAMV] = np.tile(lam, 16)
    LAM232 = np.zeros((2, 32), np.float32)
    LAM232[0, 0:8] = lam; LAM232[0, 8:16] = lam
    LAM232[1, 16:24] = lam; LAM232[1, 24:32] = lam
    cf[:2, CF_LAM232:CF_LAM232 + 32] = LAM232
    cf[8:16, CF_SHIFT32] = 0.25
    cf[24:32, CF_SHIFT32] = 0.25
    cf[0, CF_BG:CF_BG + 8] = np.asarray(bg, np.float32)
    return cf, WgPk.astype(bf16), np.ascontiguousarray(Ask).astype(bf16), SEL


# ------------------------------------------------------------- custom DVE op

def _register_frac():
    from concourse import dve_ops
    from concourse.dve_spec import Spec, Src0, C0, C1, C2, lower
    from concourse.dve_uop import DveOpSpec

    name = "FRAC0"
    for o in dve_ops.OPS:
        if o.name == name:
            return o
    u = Src0 * C0 + C1

    def _ref(in0, in1, s0, s1, imm2):
        uu = np.float32(in0 * s0 + s1)
        k = np.float32(uu + np.float32(imm2)) - np.float32(imm2)
        return np.float32(uu - k)

    spec = Spec(body=u - ((u + C2) - C2), reference=_ref)
    shas = {}
    for ver in ("v3", "v4"):
        try:
            s = DveOpSpec(name=name, opcode=0, uops=lower(spec, ver=ver), rd1_en=False)
            shas[ver] = s.sha(ver)
        except Exception:
            pass
    op = dve_ops.DveOp(name, spec, subdim=False, uops_sha=shas)
    dve_ops.OPS.append(op)
    dve_ops.CUSTOM_DVE_SPECS[name] = spec
    dve_ops._SUB_OPCODE_FOR_NAME[name] = max(dve_ops._SUB_OPCODE_FOR_NAME.values()) + 1
    return op


# ---------------------------------------------------------------- the kernel

def _build_nc():
    import concourse.bass as bass
    import concourse.mybir as mybir
    from concourse import tile, masks, bacc

    dt = mybir.dt
    AF = mybir.ActivationFunctionType
    ALU = mybir.AluOpType
    FRAC = _register_frac()

    nc = bacc.Bacc("TRN2", target_bir_lowering=False, debug=False)
    P = lambda n, s, io: nc.dram_tensor(
        n, s, dt.float32, kind="ExternalOutput" if io else "ExternalInput").ap()
    Pb = lambda n, s: nc.dram_tensor(n, s, dt.bfloat16, kind="ExternalInput").ap()

    x_d = Pb("x2b", [BL, N, D])
    boxes_d = P("boxes2", [BL, N, 4], False)
    Wq_d, Wk_d, Wv_d, Wo_d = (Pb(n, [D, D]) for n in ("Wqb", "Wkb", "Wvb", "Wob"))
    cf_d = P("constf", [128, CWF], False)
    cb_d = Pb("constb", [128, CWB])
    sel_d = nc.dram_tensor("selr", [128, 8, 128], dt.float32r, kind="ExternalInput").ap()
    out_d = P("out2", [BL, N, D], True)

    f32, f32r, bf16 = dt.float32, dt.float32r, dt.bfloat16

    with tile.TileContext(nc) as tc, ExitStack() as ctx:
        pool = ctx.enter_context(tc.tile_pool(name="resident", bufs=1))
        wk = ctx.enter_context(tc.tile_pool(name="work", bufs=2))
        wks = ctx.enter_context(tc.tile_pool(name="works", bufs=3))
        wkb = ctx.enter_context(tc.tile_pool(name="workb", bufs=3))
        ps_ex = ctx.enter_context(tc.tile_pool(name="ps_ex", bufs=2, space="PSUM"))
        ps_gps = ctx.enter_context(tc.tile_pool(name="ps_gps", bufs=2, space="PSUM"))
        ps_gt = ctx.enter_context(tc.tile_pool(name="ps_gt", bufs=2, space="PSUM"))
        ps_qkv = ctx.enter_context(tc.tile_pool(name="ps_qkv", bufs=2, space="PSUM"))

        # ---------- DMAs in priority order
        bx_b = {}
        for b in range(BL):
            bx = wk.tile([128, 2, 4], f32, tag="bx")
            nc.sync.dma_start(bx[:], boxes_d[b].rearrange("(tt p) c -> p tt c", p=128))
            bx_b[b] = bx
        cf_sb = pool.tile([128, CWF], f32)
        nc.sync.dma_start(cf_sb[:], cf_d[:])
        cb_sb = pool.tile([128, CWB], bf16)
        nc.sync.dma_start(cb_sb[:], cb_d[:])
        SELr = pool.tile([128, 8, 128], f32r)
        nc.sync.dma_start(SELr[:], sel_d[:])
        xT = pool.tile([128, 8, 2 * N], bf16)
        for b in range(BL):
            for kt in range(8):
                nc.sync.dma_start_transpose(
                    xT[:, kt, b * N:(b + 1) * N], x_d[b][:, bass.ts(kt, 128)])
        Wq_sb = pool.tile([128, 8, D], bf16)
        nc.sync.dma_start(Wq_sb[:], Wq_d.rearrange("(kt p) n -> p kt n", p=128))
        Wk_sb = pool.tile([128, 8, D], bf16)
        nc.sync.dma_start(Wk_sb[:], Wk_d.rearrange("(kt p) n -> p kt n", p=128))
        Wv_sb = pool.tile([128, 8, D], bf16)
        nc.sync.dma_start(Wv_sb[:], Wv_d.rearrange("(kt p) n -> p kt n", p=128))
        Wo_sb = pool.tile([128, 8, D], bf16)
        nc.sync.dma_start(Wo_sb[:], Wo_d.rearrange("(kt p) n -> p kt n", p=128))

        # const views
        EBC_v = cf_sb[0:8, CF_EBC:CF_EBC + 256].rearrange("p (r m) -> p r m", r=2)
        LAMV_v = cf_sb[:, CF_LAMV:CF_LAMV + 1]
        LAM232_v = cf_sb[0:2, CF_LAM232:CF_LAM232 + 32]
        SHIFT32_v = cf_sb[0:32, CF_SHIFT32:CF_SHIFT32 + 1]
        bg_v = cf_sb[0:1, CF_BG:CF_BG + 8]
        bq_v = cf_sb[:, CF_BQ:CF_BQ + 8]
        bk_v = cf_sb[:, CF_BK:CF_BK + 8]
        WgPk_v = cb_sb[:, CB_WGPK:CB_WGPK + 512].rearrange("p (b m) -> p b m", b=4)
        bvb_v = cb_sb[:, CB_BVB:CB_BVB + 1024]
        Ask_v = cb_sb[0:32, CB_ASK:CB_ASK + 256].rearrange("p (h f) -> p h f", h=8)
        bor_v = cb_sb[0:1, CB_BOR:CB_BOR + 1024]

        id_bf = pool.tile([128, 128], bf16)
        masks.make_identity(nc, id_bf[:])
        id_f32 = pool.tile([128, 128], f32)
        masks.make_identity(nc, id_f32[:])
        ONESBF = pool.tile([128, 128], bf16); nc.vector.memset(ONESBF[:], 1.0)
        ones1 = pool.tile([1, 128], bf16); nc.vector.memset(ones1[:], 1.0)

        gAT = pool.tile([128, BL, 2, H, N], bf16)   # (j, b, jh, h, i) relu'd geo^T
        qT = pool.tile([128, H, 2 * N], bf16)
        kT = pool.tile([128, H, 2 * N], bf16)
        v_sb = pool.tile([128, BL, 2, D], bf16)
        outT = pool.tile([128, H, BL, N], bf16)

        # ========== PHASE A: boxes prep (Ln region), both batches ==========
        lnd_b, lnl_b, rows_b = {}, {}, {}
        for b in range(BL):
            bx = bx_b[b]
            cols = wk.tile([128, 2, 8], f32, tag="cols")  # lnw lnh cx cy rw rh w h
            for tt in range(2):
                c = cols[:, tt, :]
                nc.vector.scalar_tensor_tensor(c[:, 6:7], bx[:, tt, 2:3], 1.0, bx[:, tt, 0:1], ALU.add, ALU.subtract)
                nc.vector.scalar_tensor_tensor(c[:, 7:8], bx[:, tt, 3:4], 1.0, bx[:, tt, 1:2], ALU.add, ALU.subtract)
                nc.vector.scalar_tensor_tensor(c[:, 2:3], bx[:, tt, 0:1], 1.0, bx[:, tt, 2:3], ALU.mult, ALU.add)
                nc.vector.tensor_scalar(c[:, 2:3], c[:, 2:3], 0.5, None, ALU.mult)
                nc.vector.scalar_tensor_tensor(c[:, 3:4], bx[:, tt, 1:2], 1.0, bx[:, tt, 3:4], ALU.mult, ALU.add)
                nc.vector.tensor_scalar(c[:, 3:4], c[:, 3:4], 0.5, None, ALU.mult)
                nc.vector.reciprocal(c[:, 4:5], c[:, 6:7])
                nc.vector.reciprocal(c[:, 5:6], c[:, 7:8])
                nc.scalar.activation(c[:, 0:2], c[:, 6:8], AF.Ln)

            rows = wk.tile([8, N], f32, tag="rows")
            rows_b[b] = rows
            for tt in range(2):
                rp = ps_gt.tile([8, 128], f32, tag="gt")
                nc.tensor.transpose(rp[:], cols[:, tt, :], id_f32[:])
                nc.scalar.copy(rows[:, bass.ts(tt, 128)], rp[:])

            cbt = wk.tile([128, 2, N], f32, tag="cb")
            for r in range(2):
                bp = ps_gt.tile([128, N], f32, tag="gt")
                nc.tensor.matmul(bp[:], EBC_v[:, r, :], rows[:], start=True, stop=True)
                nc.scalar.copy(cbt[:, r, :], bp[:])

            lnd = wk.tile([128, 2, 2, N], f32r, tag="lnd")
            lnl = wk.tile([128, 2, 2, N], f32r, tag="lnl")
            lnd_b[b], lnl_b[b] = lnd, lnl
            for it in range(2):
                for d in range(2):
                    da = wks.tile([128, N], f32, tag="da")
                    nc.vector.tensor_scalar(da[:], cbt[:, d, :], cols[:, it, 2 + d:3 + d], None, ALU.subtract)
                    nc.vector.tensor_scalar(da[:], da[:], cols[:, it, 4 + d:5 + d], None, ALU.mult)
                    nc.vector.scalar_tensor_tensor(da[:], da[:], -1.0, da[:], ALU.mult, ALU.max)
                    nc.vector.tensor_scalar(da[:], da[:], 1e-3, None, ALU.max)
                    da2 = wks.tile([128, N], f32, tag="da2")
                    nc.scalar.activation(da2[:], da[:], AF.Ln)
                    nc.vector.tensor_copy(lnd[:, d, it, :], da2[:])
                    nc.vector.tensor_sub(lnl[:, d, it, :], da2[:],
                                         lnd[:, d, it, :].bitcast(f32))

        # scheduler fence: no Sin-region op may be reordered before phase A
        tc.no_sync_barrier()

        # ========== Sin region: phase B (V33/PU) ==========
        V33_b, PU_b = {}, {}
        for b in range(BL):
            rows = rows_b[b]
            V33 = wk.tile([33, N], bf16, tag="V33")
            V33_b[b] = V33
            up = ps_gt.tile([32, N], f32, tag="gt")
            nc.tensor.matmul(up[:], LAM232_v, rows[0:2, :], start=True, stop=True)
            ur = wks.tile([32, N], f32, tag="ur")
            nc.vector._custom_dve(FRAC, out=ur[:], in0=up[:], s0=1.0, s1=SHIFT32_v, imm2=C_ROUND)
            nc.scalar.activation(V33[0:32, :], ur[:], AF.Sin, bias=0.0, scale=TWO_PI)
            nc.vector.memset(V33[32:33, :], 1.0)

            PU = wk.tile([33, 16, 128], bf16, tag="PU")
            PU_b[b] = PU
            for h in range(H):
                pp = ps_gt.tile([32, N], f32, tag="gt")
                nc.tensor.matmul(pp[:], Ask_v[:, h, :], V33[0:32, :], start=True, stop=True)
                nc.scalar.copy(PU[0:32, :, 16 * h:16 * h + 16],
                               pp[:].rearrange("p (g i) -> p g i", g=16))
                nc.vector.tensor_scalar(PU[32:33, :, 16 * h:16 * h + 16],
                                        V33[32:33, :].rearrange("p (g i) -> p g i", g=16),
                                        bg_v[0:1, h:h + 1], None, ALU.mult)

        # ---------- QKV chunk emitters (interleaved into the geo loop)
        def q_chunk(mt, which):
            W_sb, bias, dst, scale = (
                (Wq_sb, bq_v, qT, INV_SQRT_DK) if which == 'q'
                else (Wk_sb, bk_v, kT, 1.0))
            qps = ps_qkv.tile([128, 512], f32, tag="qkv")
            for kt in range(8):
                nc.tensor.matmul(qps[:], W_sb[:, kt, bass.ts(mt, 128)], xT[:, kt, :],
                                 start=(kt == 0), stop=(kt == 7))
            nc.scalar.activation(dst[:, mt, :], qps[:], AF.Identity,
                                 bias=bias[:, mt:mt + 1], scale=scale)

        def v_chunk(i):
            b, tt, chk = i // 4, (i // 2) % 2, i % 2
            vps = ps_qkv.tile([128, 512], f32, tag="qkv")
            for kt in range(8):
                nc.tensor.matmul(vps[:], xT[:, kt, b * N + tt * 128:b * N + (tt + 1) * 128],
                                 Wv_sb[:, kt, bass.ts(chk, 512)],
                                 start=(kt == 0), stop=(kt == 7))
            nc.vector.scalar_tensor_tensor(
                v_sb[:, b, tt, bass.ts(chk, 512)], vps[:], 1.0,
                bvb_v[:, bass.ts(chk, 512)], ALU.mult, ALU.add)

        chunks = ([lambda mt=mt: q_chunk(mt, 'q') for mt in range(8)]
                  + [lambda mt=mt: q_chunk(mt, 'k') for mt in range(8)]
                  + [lambda i=i: v_chunk(i) for i in range(8)])
        chunk_at = {2 + i: i for i in range(24)}   # t=2..25

        # ========== main geo loop, both batches ==========
        for t in range(BL * 16):
            b, gi = divmod(t, 16)
            lnd, lnl, V33, PU = lnd_b[b], lnl_b[b], V33_b[b], PU_b[b]
            it, gsub = divmod(gi, 8)
            ex2 = ps_ex.tile([128, 2, N], f32, tag="ex")
            nc.tensor.matmul(ex2[:], SELr[:, gsub, :],
                             lnd[:, :, it, :], start=True, stop=False)
            nc.tensor.matmul(ex2[:], SELr[:, gsub, :],
                             lnl[:, :, it, :], start=False, stop=True)
            rr4 = wkb.tile([128, 4, N], f32, tag="rr4")
            for sc in range(2):
                nc.vector._custom_dve(FRAC, out=rr4[:, 2 * sc:2 * sc + 2, :], in0=ex2[:],
                                      s0=LAMV_v, s1=0.25 * sc, imm2=C_ROUND)
            rhs = wkb.tile([128, 4, N], bf16, tag="rhs")   # (p, blk, j)
            nc.scalar.activation(rhs[:], rr4[:], AF.Sin, bias=0.0, scale=TWO_PI)
            # swapped-orientation geo matmuls: out (j, (h, i16)) lands
            # pre-transposed; relu fused into the PSUM drain
            for jh in range(2):
                gpt = ps_gps.tile([128, 128], f32, tag="gps")
                for blk in range(4):
                    nc.tensor.matmul(gpt[:], rhs[:, blk, jh * 128:(jh + 1) * 128],
                                     WgPk_v[:, blk, :], start=(blk == 0), stop=False)
                nc.tensor.matmul(gpt[:], V33[:, jh * 128:(jh + 1) * 128],
                                 PU[:, gi, :], start=False, stop=True)
                dst = gAT[:, b, jh, :, bass.ts(gi, 16)]
                src = gpt[:].rearrange("p (h i) -> p h i", h=8)
                if (gi + jh) % 2 == 0:
                    nc.vector.tensor_scalar(dst, src, 0.0, None, ALU.max)
                else:
                    nc.scalar.activation(dst, src, AF.Relu)
            if t in chunk_at:
                chunks[chunk_at[t]]()

        # ========== tail (Exp region): attention + O-projection ==========
        for b in range(BL):
            for h in range(H):
                otp = ps_ex.tile([128, N], f32, tag="ex")
                dnb = ps_gps.tile([128, N], f32, tag="gps")
                for jh in range(2):
                    stp = ps_gt.tile([128, N], f32, tag="gt")
                    nc.tensor.matmul(stp[:], kT[:, h, b * N + jh * 128:b * N + (jh + 1) * 128],
                                     qT[:, h, b * N:(b + 1) * N], start=True, stop=True)
                    pt = wks.tile([128, N], bf16, tag="pt")
                    nc.scalar.activation(pt[:], stp[:], AF.Exp)
                    un = wks.tile([128, N], bf16, tag="un")
                    nc.gpsimd.tensor_mul(un[:], pt[:], gAT[:, b, jh, h, :])
                    nc.tensor.matmul(dnb[:], ONESBF[:], un[:],
                                     start=(jh == 0), stop=(jh == 1))
                    nc.tensor.matmul(otp[:], v_sb[:, b, jh, bass.ts(h, 128)], un[:],
                                     start=(jh == 0), stop=(jh == 1))
                rcb = wks.tile([128, N], f32, tag="rcb")
                nc.vector.reciprocal(rcb[:], dnb[:])
                nc.vector.tensor_mul(outT[:, h, b, :], otp[:], rcb[:])
            # O-projection for batch b, swapped orientation: out (tok, d)
            for tt in range(2):
                for chk in range(2):
                    yps = ps_qkv.tile([128, 512], f32, tag="qkv")
                    for h in range(8):
                        nc.tensor.matmul(yps[:], outT[:, h, b, tt * 128:(tt + 1) * 128],
                                         Wo_sb[:, h, bass.ts(chk, 512)],
                                         start=(h == 0), stop=False)
                    nc.tensor.matmul(yps[:], ones1[:], bor_v[:, bass.ts(chk, 512)],
                                     start=False, stop=True)
                    ysb = wk.tile([128, 512], f32, tag="ysb")
                    nc.scalar.copy(ysb[:], yps[:])
                    nc.sync.dma_start(
                        out_d[b, tt * 128:(tt + 1) * 128, bass.ts(chk, 512)], ysb[:])

    nc.compile()
    return nc


def _get_nc():
    if "nc" not in _BUILD_CACHE:
        _BUILD_CACHE["nc"] = _build_nc()
    return _BUILD_CACHE["nc"]


def _make_in_maps(inputs):
    import concourse.mybir as mybir

    bf16 = mybir.dt.np(mybir.dt.bfloat16)
    x = np.asarray(inputs["x"], np.float32)
    boxes = np.asarray(inputs["boxes"], np.float32)
    cf, WgPk, Ask, SEL = _host_constants(inputs["Wg"], inputs["bg"], bf16)
    cf[:, CF_BQ:CF_BQ + 8] = (np.asarray(inputs["bq"], np.float32)
                              * INV_SQRT_DK).reshape(8, 128).T
    cf[:, CF_BK:CF_BK + 8] = np.asarray(inputs["bk"], np.float32).reshape(8, 128).T
    cbl = np.zeros((128, CWB), bf16)
    cbl[:, CB_WGPK:CB_WGPK + 512] = WgPk.reshape(128, 512)
    cbl[:, CB_BVB:CB_BVB + 1024] = np.tile(
        np.asarray(inputs["bv"], np.float32)[None, :], (128, 1)).astype(bf16)
    cbl[:32, CB_ASK:CB_ASK + 256] = Ask.reshape(32, 256)
    cbl[0, CB_BOR:CB_BOR + 1024] = np.asarray(inputs["bo"], np.float32).astype(bf16)
    shared = {
        "Wqb": np.asarray(inputs["Wq"], np.float32).astype(bf16),
        "Wkb": np.asarray(inputs["Wk"], np.float32).astype(bf16),
        "Wvb": np.asarray(inputs["Wv"], np.float32).astype(bf16),
        "Wob": np.asarray(inputs["Wo"], np.float32).astype(bf16),
        "constf": cf,
        "constb": cbl,
        "selr": SEL,
    }
    in_maps = []
    for c in range(NCORES):
        m = dict(shared)
        m["x2b"] = np.ascontiguousarray(x[c * BL:(c + 1) * BL]).astype(bf16)
        m["boxes2"] = np.ascontiguousarray(boxes[c * BL:(c + 1) * BL])
        in_maps.append(m)
    return in_maps


def kernel(**inputs):
    from concourse.bass_utils import run_bass_kernel_spmd

    nc = _get_nc()
    in_maps = _make_in_maps(inputs)
    res = run_bass_kernel_spmd(nc, in_maps, list(range(NCORES)))
    out = np.concatenate([res.results[c]["out2"] for c in range(NCORES)], axis=0)
    return out.astype(np.float32)


if __name__ == "__main__":
    import reference as ref
    inputs = {k: np.asarray(v) for k, v in ref.setup_inputs().items()}
    expected = np.asarray(ref.reference(**inputs))
    actual = kernel(**inputs)
    err = np.abs(actual - expected)
    scale = np.abs(expected).max()
    print(f"max_abs={err.max():.3e} scale={scale:.3f} rel={err.max()/scale:.3e}")


# revision 23
# speedup vs baseline: 1.0404x; 1.0404x over previous
"""Trainium2 Bass kernel for nn_BoxMultiHeadedAttention_81312320848177.

Self-contained: kernel(**inputs) takes FULL inputs, shards batch over 8
NeuronCores (2 batches/core), runs a Tile/Bass kernel per core, gathers.

v2 structure (per core, BL=2, N=256, D=1024, H=8, DK=128):
- DMA order: boxes, const blobs (2), Wq, xT transposes, Wk, Wv, Wo.
- Phase A (Ln region): box columns + pairwise ln-deltas (lnd/lnl f32r split).
- no_sync_barrier, then the Sin region: phase B (V33/PU) + 32 (b,gi) geo
  iterations with the QKV projection chunks interleaved (q@t2+, k@t10+, v@t18+).
- Geo per gi: SEL-expansion matmuls (f32r), FRAC range reduction (DVE custom),
  Sin (ACT), packed WgPk/PU matmuls, relu on Pool, transpose, gAT copies
  split DVE/Pool.
- Tail (Exp region): attention per (b,h): scores, exp, geo-mul, denom+attnV
  matmuls, reciprocal-normalize; O-projection in swapped orientation
  (tokens on partitions) so the output needs no transposes; bias via rank-1
  matmul; direct DMA store.
"""
import sys
sys.path.insert(0, '/opt/trn_rl_repo')

import numpy as np
from contextlib import ExitStack

B, N, D, H, DK = 16, 256, 1024, 8, 128
BL = 2                 # batches per core
NCORES = 8
WAVE_LEN = 1000.0
C_ROUND = float(1.5 * 2**23)
TWO_PI = float(2 * np.pi)
INV_SQRT_DK = float(1.0 / np.sqrt(DK))

# const blob column layout (f32 blob: (128, CWF); bf16 blob: (128, CWB))
CF_SEL = 0          # (128, 8, 128) -> cols [0, 1024)
CF_EBC = 1024       # (8, 2, 128) on partitions 0-7 -> cols [1024, 1280)
CF_LAMV = 1280      # (128, 1)
CF_LAM232 = 1281    # (2, 32) partitions 0-1 -> cols [1281, 1313)
CF_SHIFT32 = 1313   # (32, 1) partitions 0-31
CF_BG = 1314        # (1, 8) partition 0
CF_BQ = 1322        # (128, 8)
CF_BK = 1330        # (128, 8)
CWF = 1338

CB_WGPK = 0         # (128, 4, 128) -> cols [0, 512)
CB_BVB = 512        # (128, 1024) -> cols [512, 1536)
CB_ASK = 1536       # (32, 8, 32) partitions 0-31 -> cols [1536, 1792)
CB_BOR = 1792       # (1, 1024) partition 0
CWB = 2816

_BUILD_CACHE = {}


# ------------------------------------------------------------------ host prep

def _lam():
    f = np.arange(8, dtype=np.float64)
    return (100.0 / (2 * np.pi) * WAVE_LEN ** (-f / 8)).astype(np.float32)


def _host_constants(Wg, bg, bf16):
    """Pack all small constants into one f32 blob and one bf16 blob."""
    lam = _lam()
    Wg = np.asarray(Wg, np.float32)

    blocks = [(0, 0), (1, 8), (2, 32), (3, 40)]  # (blk, col0): xs, ys, xc, yc
    WgPk = np.zeros((128, 4, 128), np.float32)
    for blk, col0 in blocks:
        for h in range(H):
            for i16 in range(16):
                m = 16 * h + i16
                for fi in range(8):
                    k = i16 * 8 + fi
                    WgPk[k, blk, m] = Wg[h, col0 + fi]

    A = np.zeros((H, 32, 32), np.float32)
    for h in range(H):
        wsw, wcw = Wg[h, 16:24], Wg[h, 48:56]
        wsh, wch = Wg[h, 24:32], Wg[h, 56:64]
        for fi in range(8):
            A[h, fi, 8 + fi] += wsw[fi]
            A[h, 8 + fi, fi] += -wsw[fi]
            A[h, 8 + fi, 8 + fi] += wcw[fi]
            A[h, fi, fi] += wcw[fi]
            A[h, 16 + fi, 24 + fi] += wsh[fi]
            A[h, 24 + fi, 16 + fi] += -wsh[fi]
            A[h, 24 + fi, 24 + fi] += wch[fi]
            A[h, 16 + fi, 16 + fi] += wch[fi]
    Ask = A.transpose(1, 0, 2)  # (32 k, 8 h, 32 f'): Ask[k, h, f'] = A[h, k, f']

    SEL = np.zeros((128, 8, 128), np.float32)
    for gsub in range(8):
        for ii in range(16):
            for fi in range(8):
                SEL[16 * gsub + ii, gsub, ii * 8 + fi] = 1.0

    EBC = np.zeros((8, 2, 128), np.float32)
    EBC[2, 0, :] = 1.0
    EBC[3, 1, :] = 1.0

    cf = np.zeros((128, CWF), np.float32)
    cf[:, CF_SEL:CF_SEL + 1024] = SEL.reshape(128, 1024)
    cf[:8, CF_EBC:CF_EBC + 256] = EBC.reshape(8, 256)
    cf[:, CF_LAMV] = np.tile(lam, 16)
    LAM232 = np.zeros((2, 32), np.float32)
    LAM232[0, 0:8] = lam; LAM232[0, 8:16] = lam
    LAM232[1, 16:24] = lam; LAM232[1, 24:32] = lam
    cf[:2, CF_LAM232:CF_LAM232 + 32] = LAM232
    cf[8:16, CF_SHIFT32] = 0.25
    cf[24:32, CF_SHIFT32] = 0.25
    cf[0, CF_BG:CF_BG + 8] = np.asarray(bg, np.float32)
    return cf, WgPk.astype(bf16), np.ascontiguousarray(Ask).astype(bf16), SEL


# ------------------------------------------------------------- custom DVE op

def _register_frac():
    from concourse import dve_ops
    from concourse.dve_spec import Spec, Src0, C0, C1, C2, lower
    from concourse.dve_uop import DveOpSpec

    name = "FRAC0"
    for o in dve_ops.OPS:
        if o.name == name:
            return o
    u = Src0 * C0 + C1

    def _ref(in0, in1, s0, s1, imm2):
        uu = np.float32(in0 * s0 + s1)
        k = np.float32(uu + np.float32(imm2)) - np.float32(imm2)
        return np.float32(uu - k)

    spec = Spec(body=u - ((u + C2) - C2), reference=_ref)
    shas = {}
    for ver in ("v3", "v4"):
        try:
            s = DveOpSpec(name=name, opcode=0, uops=lower(spec, ver=ver), rd1_en=False)
            shas[ver] = s.sha(ver)
        except Exception:
            pass
    op = dve_ops.DveOp(name, spec, subdim=False, uops_sha=shas)
    dve_ops.OPS.append(op)
    dve_ops.CUSTOM_DVE_SPECS[name] = spec
    dve_ops._SUB_OPCODE_FOR_NAME[name] = max(dve_ops._SUB_OPCODE_FOR_NAME.values()) + 1
    return op


def _register_absclip():
    from concourse import dve_ops
    from concourse.dve_spec import Spec, Src0, C0, Zero, lower, maxx
    from concourse.dve_uop import DveOpSpec

    name = "ABSCLIP0"
    for o in dve_ops.OPS:
        if o.name == name:
            return o

    def _ref(in0, in1, s0, s1, imm2):
        return np.float32(np.maximum(np.abs(np.float32(in0)), np.float32(s0)))

    spec = Spec(body=maxx(maxx(Src0, Zero - Src0), C0), reference=_ref)
    shas = {}
    for ver in ("v3", "v4"):
        try:
            s = DveOpSpec(name=name, opcode=0, uops=lower(spec, ver=ver), rd1_en=False)
            shas[ver] = s.sha(ver)
        except Exception:
            pass
    op = dve_ops.DveOp(name, spec, subdim=False, uops_sha=shas)
    dve_ops.OPS.append(op)
    dve_ops.CUSTOM_DVE_SPECS[name] = spec
    dve_ops._SUB_OPCODE_FOR_NAME[name] = max(dve_ops._SUB_OPCODE_FOR_NAME.values()) + 1
    return op


# ---------------------------------------------------------------- the kernel

def _build_nc():
    import concourse.bass as bass
    import concourse.mybir as mybir
    from concourse import tile, masks, bacc

    dt = mybir.dt
    AF = mybir.ActivationFunctionType
    ALU = mybir.AluOpType
    FRAC = _register_frac()
    ABSCLIP = _register_absclip()

    nc = bacc.Bacc("TRN2", target_bir_lowering=False, debug=False)
    P = lambda n, s, io: nc.dram_tensor(
        n, s, dt.float32, kind="ExternalOutput" if io else "ExternalInput").ap()
    Pb = lambda n, s: nc.dram_tensor(n, s, dt.bfloat16, kind="ExternalInput").ap()

    x_d = Pb("x2b", [BL, N, D])
    boxes_d = P("boxes2", [BL, N, 4], False)
    Wq_d, Wk_d, Wv_d, Wo_d = (Pb(n, [D, D]) for n in ("Wqb", "Wkb", "Wvb", "Wob"))
    cf_d = P("constf", [128, CWF], False)
    cb_d = Pb("constb", [128, CWB])
    sel_d = nc.dram_tensor("selr", [128, 8, 128], dt.float32r, kind="ExternalInput").ap()
    out_d = P("out2", [BL, N, D], True)

    f32, f32r, bf16 = dt.float32, dt.float32r, dt.bfloat16

    with tile.TileContext(nc) as tc, ExitStack() as ctx:
        pool = ctx.enter_context(tc.tile_pool(name="resident", bufs=1))
        wk = ctx.enter_context(tc.tile_pool(name="work", bufs=2))
        wks = ctx.enter_context(tc.tile_pool(name="works", bufs=3))
        wkb = ctx.enter_context(tc.tile_pool(name="workb", bufs=3))
        ps_ex = ctx.enter_context(tc.tile_pool(name="ps_ex", bufs=2, space="PSUM"))
        ps_gps = ctx.enter_context(tc.tile_pool(name="ps_gps", bufs=2, space="PSUM"))
        ps_gt = ctx.enter_context(tc.tile_pool(name="ps_gt", bufs=2, space="PSUM"))
        ps_qkv = ctx.enter_context(tc.tile_pool(name="ps_qkv", bufs=2, space="PSUM"))

        # ---------- DMAs in priority order
        bx_b = {}
        for b in range(BL):
            bx = wk.tile([128, 2, 4], f32, tag="bx")
            nc.sync.dma_start(bx[:], boxes_d[b].rearrange("(tt p) c -> p tt c", p=128))
            bx_b[b] = bx
        cf_sb = pool.tile([128, CWF], f32)
        nc.sync.dma_start(cf_sb[:], cf_d[:])
        cb_sb = pool.tile([128, CWB], bf16)
        nc.sync.dma_start(cb_sb[:], cb_d[:])
        SELr = pool.tile([128, 8, 128], f32r)
        nc.sync.dma_start(SELr[:], sel_d[:])
        xT = pool.tile([128, 8, 2 * N], bf16)
        for b in range(BL):
            for kt in range(8):
                nc.sync.dma_start_transpose(
                    xT[:, kt, b * N:(b + 1) * N], x_d[b][:, bass.ts(kt, 128)])
        Wq_sb = pool.tile([128, 8, D], bf16)
        nc.sync.dma_start(Wq_sb[:], Wq_d.rearrange("(kt p) n -> p kt n", p=128))
        Wk_sb = pool.tile([128, 8, D], bf16)
        nc.sync.dma_start(Wk_sb[:], Wk_d.rearrange("(kt p) n -> p kt n", p=128))
        Wv_sb = pool.tile([128, 8, D], bf16)
        nc.sync.dma_start(Wv_sb[:], Wv_d.rearrange("(kt p) n -> p kt n", p=128))
        Wo_sb = pool.tile([128, 8, D], bf16)
        nc.sync.dma_start(Wo_sb[:], Wo_d.rearrange("(kt p) n -> p kt n", p=128))

        # const views
        EBC_v = cf_sb[0:8, CF_EBC:CF_EBC + 256].rearrange("p (r m) -> p r m", r=2)
        LAMV_v = cf_sb[:, CF_LAMV:CF_LAMV + 1]
        LAM232_v = cf_sb[0:2, CF_LAM232:CF_LAM232 + 32]
        SHIFT32_v = cf_sb[0:32, CF_SHIFT32:CF_SHIFT32 + 1]
        bg_v = cf_sb[0:1, CF_BG:CF_BG + 8]
        bq_v = cf_sb[:, CF_BQ:CF_BQ + 8]
        bk_v = cf_sb[:, CF_BK:CF_BK + 8]
        WgPk_v = cb_sb[:, CB_WGPK:CB_WGPK + 512].rearrange("p (b m) -> p b m", b=4)
        bvb_v = cb_sb[:, CB_BVB:CB_BVB + 1024]
        Ask_v = cb_sb[0:32, CB_ASK:CB_ASK + 256].rearrange("p (h f) -> p h f", h=8)
        bor_v = cb_sb[0:1, CB_BOR:CB_BOR + 1024]

        id_bf = pool.tile([128, 128], bf16)
        masks.make_identity(nc, id_bf[:])
        id_f32 = pool.tile([128, 128], f32)
        masks.make_identity(nc, id_f32[:])
        ONESBF = pool.tile([128, 128], bf16); nc.vector.memset(ONESBF[:], 1.0)
        ones1 = pool.tile([1, 128], bf16); nc.vector.memset(ones1[:], 1.0)

        gAT = pool.tile([128, BL, 2, H, N], bf16)   # (j, b, jh, h, i) relu'd geo^T
        qT = pool.tile([128, H, 2 * N], bf16)
        kT = pool.tile([128, H, 2 * N], bf16)
        v_sb = pool.tile([128, BL, 2, D], bf16)
        outT = pool.tile([128, H, BL, N], bf16)

        # ========== PHASE A: boxes prep (Ln region), both batches ==========
        lnd_b, lnl_b, rows_b = {}, {}, {}
        for b in range(BL):
            bx = bx_b[b]
            cols = wk.tile([128, 2, 8], f32, tag="cols")  # lnw lnh cx cy rw rh w h
            for tt in range(2):
                c = cols[:, tt, :]
                nc.vector.scalar_tensor_tensor(c[:, 6:7], bx[:, tt, 2:3], 1.0, bx[:, tt, 0:1], ALU.add, ALU.subtract)
                nc.vector.scalar_tensor_tensor(c[:, 7:8], bx[:, tt, 3:4], 1.0, bx[:, tt, 1:2], ALU.add, ALU.subtract)
                nc.vector.scalar_tensor_tensor(c[:, 2:3], bx[:, tt, 0:1], 1.0, bx[:, tt, 2:3], ALU.mult, ALU.add)
                nc.vector.tensor_scalar(c[:, 2:3], c[:, 2:3], 0.5, None, ALU.mult)
                nc.vector.scalar_tensor_tensor(c[:, 3:4], bx[:, tt, 1:2], 1.0, bx[:, tt, 3:4], ALU.mult, ALU.add)
                nc.vector.tensor_scalar(c[:, 3:4], c[:, 3:4], 0.5, None, ALU.mult)
                nc.vector.reciprocal(c[:, 4:5], c[:, 6:7])
                nc.vector.reciprocal(c[:, 5:6], c[:, 7:8])
                nc.scalar.activation(c[:, 0:2], c[:, 6:8], AF.Ln)

            rows = wk.tile([8, N], f32, tag="rows")
            rows_b[b] = rows
            for tt in range(2):
                rp = ps_gt.tile([8, 128], f32, tag="gt")
                nc.tensor.transpose(rp[:], cols[:, tt, :], id_f32[:])
                nc.scalar.copy(rows[:, bass.ts(tt, 128)], rp[:])

            bp = ps_ex.tile([128, 2, N], f32, tag="ex")
            for r in range(2):
                nc.tensor.matmul(bp[:, r, :], EBC_v[:, r, :], rows[:], start=True, stop=True)

            lnd = wk.tile([128, 2, 2, N], f32r, tag="lnd")
            lnl = wk.tile([128, 2, 2, N], f32r, tag="lnl")
            lnd_b[b], lnl_b[b] = lnd, lnl
            for it in range(2):
                for d in range(2):
                    da = wks.tile([128, N], f32, tag="da")
                    nc.vector.tensor_scalar(da[:], bp[:, d, :], cols[:, it, 2 + d:3 + d],
                                            cols[:, it, 4 + d:5 + d], ALU.subtract, ALU.mult)
                    da2 = wks.tile([128, N], f32, tag="da2")
                    nc.vector._custom_dve(ABSCLIP, out=da2[:], in0=da[:], s0=1e-3, s1=0.0, imm2=0.0)
                    da3 = wks.tile([128, N], f32, tag="da3")
                    nc.scalar.activation(da3[:], da2[:], AF.Ln)
                    nc.vector.tensor_copy(lnd[:, d, it, :], da3[:])
                    nc.gpsimd.tensor_sub(lnl[:, d, it, :], da3[:],
                                         lnd[:, d, it, :].bitcast(f32))

        # scheduler fence: no Sin-region op may be reordered before phase A
        tc.no_sync_barrier()

        # ========== Sin region: phase B (V33/PU) ==========
        V33_b, PU_b = {}, {}
        for b in range(BL):
            rows = rows_b[b]
            V33 = wk.tile([33, N], bf16, tag="V33")
            V33_b[b] = V33
            up = ps_gt.tile([32, N], f32, tag="gt")
            nc.tensor.matmul(up[:], LAM232_v, rows[0:2, :], start=True, stop=True)
            ur = wks.tile([32, N], f32, tag="ur")
            nc.vector._custom_dve(FRAC, out=ur[:], in0=up[:], s0=1.0, s1=SHIFT32_v, imm2=C_ROUND)
            nc.scalar.activation(V33[0:32, :], ur[:], AF.Sin, bias=0.0, scale=TWO_PI)
            nc.vector.memset(V33[32:33, :], 1.0)

            PU = wk.tile([33, 16, 128], bf16, tag="PU")
            PU_b[b] = PU
            for h in range(H):
                pp = ps_gt.tile([32, N], f32, tag="gt")
                nc.tensor.matmul(pp[:], Ask_v[:, h, :], V33[0:32, :], start=True, stop=True)
                nc.scalar.copy(PU[0:32, :, 16 * h:16 * h + 16],
                               pp[:].rearrange("p (g i) -> p g i", g=16))
                nc.vector.tensor_scalar(PU[32:33, :, 16 * h:16 * h + 16],
                                        V33[32:33, :].rearrange("p (g i) -> p g i", g=16),
                                        bg_v[0:1, h:h + 1], None, ALU.mult)

        # ---------- QKV chunk emitters (interleaved into the geo loop)
        def q_chunk(mt, which):
            W_sb, bias, dst, scale = (
                (Wq_sb, bq_v, qT, INV_SQRT_DK) if which == 'q'
                else (Wk_sb, bk_v, kT, 1.0))
            qps = ps_qkv.tile([128, 512], f32, tag="qkv")
            for kt in range(8):
                nc.tensor.matmul(qps[:], W_sb[:, kt, bass.ts(mt, 128)], xT[:, kt, :],
                                 start=(kt == 0), stop=(kt == 7))
            nc.scalar.activation(dst[:, mt, :], qps[:], AF.Identity,
                                 bias=bias[:, mt:mt + 1], scale=scale)

        def v_chunk(i):
            b, tt, chk = i // 4, (i // 2) % 2, i % 2
            vps = ps_qkv.tile([128, 512], f32, tag="qkv")
            for kt in range(8):
                nc.tensor.matmul(vps[:], xT[:, kt, b * N + tt * 128:b * N + (tt + 1) * 128],
                                 Wv_sb[:, kt, bass.ts(chk, 512)],
                                 start=(kt == 0), stop=(kt == 7))
            nc.vector.scalar_tensor_tensor(
                v_sb[:, b, tt, bass.ts(chk, 512)], vps[:], 1.0,
                bvb_v[:, bass.ts(chk, 512)], ALU.mult, ALU.add)

        chunks = ([lambda mt=mt: q_chunk(mt, 'q') for mt in range(8)]
                  + [lambda mt=mt: q_chunk(mt, 'k') for mt in range(8)]
                  + [lambda i=i: v_chunk(i) for i in range(8)])
        chunk_at = {2 + i: i for i in range(24)}   # t=2..25

        # ========== main geo loop, both batches ==========
        for t in range(BL * 16):
            b, gi = divmod(t, 16)
            lnd, lnl, V33, PU = lnd_b[b], lnl_b[b], V33_b[b], PU_b[b]
            it, gsub = divmod(gi, 8)
            ex2 = ps_ex.tile([128, 2, N], f32, tag="ex")
            nc.tensor.matmul(ex2[:], SELr[:, gsub, :],
                             lnd[:, :, it, :], start=True, stop=False)
            nc.tensor.matmul(ex2[:], SELr[:, gsub, :],
                             lnl[:, :, it, :], start=False, stop=True)
            rr4 = wkb.tile([128, 4, N], f32, tag="rr4")
            for sc in range(2):
                nc.vector._custom_dve(FRAC, out=rr4[:, 2 * sc:2 * sc + 2, :], in0=ex2[:],
                                      s0=LAMV_v, s1=0.25 * sc, imm2=C_ROUND)
            rhs = wkb.tile([128, 4, N], bf16, tag="rhs")   # (p, blk, j)
            nc.scalar.activation(rhs[:], rr4[:], AF.Sin, bias=0.0, scale=TWO_PI)
            # swapped-orientation geo matmuls: out (j, (h, i16)) lands
            # pre-transposed; relu fused into the PSUM drain
            for jh in range(2):
                gpt = ps_gps.tile([128, 128], f32, tag="gps")
                for blk in range(4):
                    nc.tensor.matmul(gpt[:], rhs[:, blk, jh * 128:(jh + 1) * 128],
                                     WgPk_v[:, blk, :], start=(blk == 0), stop=False)
                nc.tensor.matmul(gpt[:], V33[:, jh * 128:(jh + 1) * 128],
                                 PU[:, gi, :], start=False, stop=True)
                dst = gAT[:, b, jh, :, bass.ts(gi, 16)]
                src = gpt[:].rearrange("p (h i) -> p h i", h=8)
                if (gi + jh) % 2 == 0:
                    nc.vector.tensor_scalar(dst, src, 0.0, None, ALU.max)
                else:
                    nc.scalar.activation(dst, src, AF.Relu)
            if t in chunk_at:
                chunks[chunk_at[t]]()

        # ========== tail (Exp region): attention + O-projection ==========
        for b in range(BL):
            for h in range(H):
                stp2 = ps_ex.tile([128, 2, N], f32, tag="ex")
                for jh in range(2):
                    nc.tensor.matmul(stp2[:, jh, :],
                                     kT[:, h, b * N + jh * 128:b * N + (jh + 1) * 128],
                                     qT[:, h, b * N:(b + 1) * N], start=True, stop=True)
                pt2 = wks.tile([128, 2, N], bf16, tag="pt")
                nc.scalar.activation(pt2[:], stp2[:], AF.Exp)
                un2 = wks.tile([128, 2, N], bf16, tag="un")
                nc.vector.tensor_mul(un2[:], pt2[:], gAT[:, b, :, h, :])
                otp = ps_gps.tile([128, N], f32, tag="gps")
                dnb = ps_gt.tile([128, N], f32, tag="gt")
                for jh in range(2):
                    nc.tensor.matmul(dnb[:], ONESBF[:], un2[:, jh, :],
                                     start=(jh == 0), stop=(jh == 1))
                    nc.tensor.matmul(otp[:], v_sb[:, b, jh, bass.ts(h, 128)], un2[:, jh, :],
                                     start=(jh == 0), stop=(jh == 1))
                rcb = wks.tile([128, N], f32, tag="rcb")
                nc.vector.reciprocal(rcb[:], dnb[:])
                nc.vector.tensor_mul(outT[:, h, b, :], otp[:], rcb[:])
            # O-projection for batch b, swapped orientation: out (tok, d)
            for tt in range(2):
                for chk in range(2):
                    yps = ps_qkv.tile([128, 512], f32, tag="qkv")
                    for h in range(8):
                        nc.tensor.matmul(yps[:], outT[:, h, b, tt * 128:(tt + 1) * 128],
                                         Wo_sb[:, h, bass.ts(chk, 512)],
                                         start=(h == 0), stop=False)
                    nc.tensor.matmul(yps[:], ones1[:], bor_v[:, bass.ts(chk, 512)],
                                     start=False, stop=True)
                    ysb = wk.tile([128, 512], f32, tag="ysb")
                    nc.scalar.copy(ysb[:], yps[:])
                    nc.sync.dma_start(
                        out_d[b, tt * 128:(tt + 1) * 128, bass.ts(chk, 512)], ysb[:])

    nc.compile()
    return nc


def _get_nc():
    if "nc" not in _BUILD_CACHE:
        _BUILD_CACHE["nc"] = _build_nc()
    return _BUILD_CACHE["nc"]


def _make_in_maps(inputs):
    import concourse.mybir as mybir

    bf16 = mybir.dt.np(mybir.dt.bfloat16)
    x = np.asarray(inputs["x"], np.float32)
    boxes = np.asarray(inputs["boxes"], np.float32)
    cf, WgPk, Ask, SEL = _host_constants(inputs["Wg"], inputs["bg"], bf16)
    cf[:, CF_BQ:CF_BQ + 8] = (np.asarray(inputs["bq"], np.float32)
                              * INV_SQRT_DK).reshape(8, 128).T
    cf[:, CF_BK:CF_BK + 8] = np.asarray(inputs["bk"], np.float32).reshape(8, 128).T
    cbl = np.zeros((128, CWB), bf16)
    cbl[:, CB_WGPK:CB_WGPK + 512] = WgPk.reshape(128, 512)
    cbl[:, CB_BVB:CB_BVB + 1024] = np.tile(
        np.asarray(inputs["bv"], np.float32)[None, :], (128, 1)).astype(bf16)
    cbl[:32, CB_ASK:CB_ASK + 256] = Ask.reshape(32, 256)
    cbl[0, CB_BOR:CB_BOR + 1024] = np.asarray(inputs["bo"], np.float32).astype(bf16)
    shared = {
        "Wqb": np.asarray(inputs["Wq"], np.float32).astype(bf16),
        "Wkb": np.asarray(inputs["Wk"], np.float32).astype(bf16),
        "Wvb": np.asarray(inputs["Wv"], np.float32).astype(bf16),
        "Wob": np.asarray(inputs["Wo"], np.float32).astype(bf16),
        "constf": cf,
        "constb": cbl,
        "selr": SEL,
    }
    in_maps = []
    for c in range(NCORES):
        m = dict(shared)
        m["x2b"] = np.ascontiguousarray(x[c * BL:(c + 1) * BL]).astype(bf16)
        m["boxes2"] = np.ascontiguousarray(boxes[c * BL:(c + 1) * BL])
        in_maps.append(m)
    return in_maps


def kernel(**inputs):
    from concourse.bass_utils import run_bass_kernel_spmd

    nc = _get_nc()
    in_maps = _make_in_maps(inputs)
    res = run_bass_kernel_spmd(nc, in_maps, list(range(NCORES)))
    out = np.concatenate([res.results[c]["out2"] for c in range(NCORES)], axis=0)
    return out.astype(np.float32)


if __name__ == "__main__":
    import reference as ref
    inputs = {k: np.asarray(v) for k, v in ref.setup_inputs().items()}
    expected = np.asarray(ref.reference(**inputs))
    actual = kernel(**inputs)
    err = np.abs(actual - expected)
    scale = np.abs(expected).max()
    print(f"max_abs={err.max():.3e} scale={scale:.3f} rel={err.max()/scale:.3e}")
